# revision 1
# baseline (speedup 1.0000x reference)
"""Trainium2 Bass kernel for nn_Attention_LoRA (Swin-style attention w/ LoRA + rel-pos bias).

Strategy:
  - Data-parallel over batch: 64 batches -> 8 cores x 8 batches.
  - Host-side prep (pure layout / constant folding, no per-token compute):
      * LoRA adapters folded into qkv weight:  W_eff = qkv_w + B @ A  (per q/k/v)
      * attention scale folded into q weights & bias
      * weights pre-transposed; v-weights head-interleaved with a ones column so
        the PV matmul also produces the softmax denominator
      * rel-pos bias gathered + exponentiated: softmax(S+rpb) uses exp(S)*exp(rpb)
      * x pre-transposed to [B, C, N] and token-padded to 256
  - On-device, per batch, everything stays "transposed" (features/keys on
    partitions) so NO on-chip transposes are needed:
      qkT[c,t]  = Wqk @ xT                       (12x6 matmuls, fp32r)
      v[t,c']   = x @ WvT_ext                    (token-major, ones col + bias row)
      ST[k,q]   = kT.T @ qT                      (per head, keys on partitions)
      PT[k,q]   = exp(ST) * exp_rpbT[h]
      OT_ext    = v_ext.T @ PT   -> rows 0:64 = unnormalized out, row 64 = sum_k P
      aoT       = OT[0:64] * (1/l)  (GpSimd partition-broadcast of 1/l)
      y[t,c]    = aoT.T @ projWT + (proj_b + proj_w @ v_bias)
  Measured on trn2 (8 cores, axon): ~490-520 us absolute (direct method) per
  core for its 8-batch share (paired A/B ordering: 88 us faster than the v7
  variant of this kernel),
  scaled absmax error vs fp32 reference ~2.9e-4 (fp32r matmuls).
"""
import os
from contextlib import ExitStack

import numpy as np

import concourse.bacc as bacc
import concourse.mybir as mybir
import concourse.tile as tile
from concourse import bass_utils

B, NT, C, H, WS, RANK = 64, 197, 768, 12, 14, 24
HD = C // H
SCALE = HD ** -0.5
NCORES = 8
BPC = B // NCORES          # batches per core
TP = 256                   # padded token count
KC = C // 128              # 6 contraction chunks
NCO = (2 * C) // 128       # 12 q+k output chunks
VW = H * (HD + 1)          # 780: v head-blocks of 65 (64 v cols + ones col)
VH = VW // 2               # 390
PH = C // 2                # 384

MM_MODE = os.environ.get("MM_MODE", "f32r")   # f32r | bf16 | f32
PERF_REPS = int(os.environ.get("PERF_REPS", "0"))

_prog_cache = {}


def _mm_dt():
    return {
        "f32r": mybir.dt.float32r,
        "bf16": mybir.dt.bfloat16,
        "f32": mybir.dt.float32,
    }[MM_MODE]


def _mm_np():
    if MM_MODE == "bf16":
        import ml_dtypes
        return ml_dtypes.bfloat16
    return np.float32


def _bf16():
    import ml_dtypes
    return ml_dtypes.bfloat16


def _build_program():
    key = (MM_MODE, PERF_REPS)
    if key in _prog_cache:
        return _prog_cache[key]

    f32 = mybir.dt.float32
    mdt = _mm_dt()
    Exp = mybir.ActivationFunctionType.Exp

    nc = bacc.Bacc("TRN2", target_bir_lowering=False, debug=False,
                   num_devices=NCORES)
    xt_d = nc.dram_tensor("xt", [BPC // 2, C, 2 * TP], mdt,
                          kind="ExternalInput").ap()
    wqk_d = nc.dram_tensor("wqk", [C, 2 * C], mdt, kind="ExternalInput").ap()
    bqk_d = nc.dram_tensor("bqk", [128, NCO], f32, kind="ExternalInput").ap()
    wv_d = nc.dram_tensor("wv", [C + 1, VW], mdt, kind="ExternalInput").ap()
    ones_d = nc.dram_tensor("ones", [1, TP], mdt, kind="ExternalInput").ap()
    pw_d = nc.dram_tensor("pw", [C, C], mdt, kind="ExternalInput").ap()
    pb_d = nc.dram_tensor("pb", [1, C], f32, kind="ExternalInput").ap()
    erpb_d = nc.dram_tensor("erpb", [H, 128, 2 * TP], mybir.dt.float16,
                            kind="ExternalInput").ap()
    y_d = nc.dram_tensor("y", [BPC, NT, C], f32, kind="ExternalOutput").ap()

    with tile.TileContext(nc) as tc, ExitStack() as ctx, \
            nc.allow_low_precision("fp32r/bf16 matmul inputs by design"):
        consts = ctx.enter_context(tc.tile_pool(name="consts", bufs=1))
        xp = ctx.enter_context(tc.tile_pool(name="xp", bufs=1))
        qkp = ctx.enter_context(tc.tile_pool(name="qkp", bufs=1))
        vp = ctx.enter_context(tc.tile_pool(name="vp", bufs=1))
        ep = ctx.enter_context(tc.tile_pool(name="ep", bufs=2))
        ptp = ctx.enter_context(tc.tile_pool(name="ptp", bufs=3))
        lrp = ctx.enter_context(tc.tile_pool(name="lrp", bufs=3))
        rfp = ctx.enter_context(tc.tile_pool(name="rfp", bufs=3))
        aop = ctx.enter_context(tc.tile_pool(name="aop", bufs=2))
        yp = ctx.enter_context(tc.tile_pool(name="yp", bufs=2))
        psA = ctx.enter_context(tc.tile_pool(name="psA", bufs=2, space="PSUM"))
        psS = ctx.enter_context(tc.tile_pool(name="psS", bufs=3, space="PSUM"))
        psO = ctx.enter_context(tc.tile_pool(name="psO", bufs=3, space="PSUM"))

        # ---- constants ----
        wqk_sb = []
        wv_sb = []
        pw_sb = []
        for kc in range(KC):
            t = consts.tile([128, 2 * C], mdt, tag=f"wqk{kc}")
            nc.sync.dma_start(out=t, in_=wqk_d[kc * 128:(kc + 1) * 128, :])
            wqk_sb.append(t)
        for kc in range(KC):
            t = consts.tile([128, VW], mdt, tag=f"wv{kc}")
            nc.sync.dma_start(out=t, in_=wv_d[kc * 128:(kc + 1) * 128, :])
            wv_sb.append(t)
        for kc in range(KC):
            t = consts.tile([128, C], mdt, tag=f"pw{kc}")
            nc.sync.dma_start(out=t, in_=pw_d[kc * 128:(kc + 1) * 128, :])
            pw_sb.append(t)
        wv_bias = consts.tile([1, VW], mdt, tag="wvb")
        nc.sync.dma_start(out=wv_bias, in_=wv_d[C:C + 1, :])
        ones_sb = consts.tile([1, TP], mdt, tag="ones")
        nc.sync.dma_start(out=ones_sb, in_=ones_d)
        pb_full = consts.tile([128, C], f32, tag="pbf")
        import concourse.bass as bass
        nc.sync.dma_start(out=pb_full, in_=bass.AP(
            tensor=pb_d.tensor, offset=pb_d.offset,
            ap=[[0, 128]] + list(pb_d.ap[1:])))
        bqk_sb = consts.tile([128, NCO], f32, tag="bqk")
        nc.sync.dma_start(out=bqk_sb, in_=bqk_d)
        erpb_sb = {}
        for h in range(H):
            t = consts.tile([128, 2 * TP], mybir.dt.float16, tag=f"erpb{h}")
            nc.sync.dma_start(out=t, in_=erpb_d[h, :, :])
            erpb_sb[h] = t

        KT_SZ = [128, NT - 128]  # 128, 69

        PAIRS = BPC // 2
        co_order = [c for pr in zip(range(KC), range(KC, NCO)) for c in pr]

        def build_A(p):
            """Emit pair p's xt DMAs now; return (qk_sb, v_sbs, thunks).
            Each thunk emits one qk or v matmul group when invoked, so the
            caller can spread pair p's dense groups between the previous
            pair's latency-bound attention units. Tiles use pair-parity tag
            namespaces (bufs=1 per tag) so consecutive pairs never rotate
            through the same slot."""
            par = p % 2
            xt_sb = []
            for kc in range(KC):
                t = xp.tile([128, 2 * TP], mdt, tag=f"x{par}_{kc}",
                            name=f"x{par}_{kc}")
                nc.sync.dma_start(out=t, in_=xt_d[p, kc * 128:(kc + 1) * 128, :])
                xt_sb.append(t)
            qk_sb = [[None] * NCO, [None] * NCO]

            def qk_thunk(co):
                ps = psA.tile([128, 2 * TP], f32, tag="mm", name="mm")
                for kc in range(KC):
                    nc.tensor.matmul(
                        ps, wqk_sb[kc][:, co * 128:(co + 1) * 128], xt_sb[kc],
                        start=(kc == 0), stop=(kc == KC - 1))
                for b01 in range(2):
                    qk = qkp.tile([128, TP], mdt, tag=f"qk{par}_{b01}_{co}",
                                  name=f"qk{par}_{b01}_{co}")
                    nc.scalar.add(qk, ps[:, b01 * TP:(b01 + 1) * TP],
                                  add=bqk_sb[:, co:co + 1])
                    qk_sb[b01][co] = qk

            thunks = [lambda co=co: qk_thunk(co) for co in co_order]
            return qk_sb, xt_sb, thunks

        def emit_V(xt_sb):
            """Emit both batches' v passes (token-major) for the current pair."""
            v_sbs = [[None, None], [None, None]]
            for b01 in range(2):
                xo = b01 * TP
                for tt in range(2):
                    v = vp.tile([128, VW], mdt, tag=f"v{b01}_{tt}",
                                name=f"v{b01}_{tt}")
                    for half in range(2):
                        ps = psA.tile([128, VH], f32, tag="mm", name="mm")
                        for kc in range(KC):
                            nc.tensor.matmul(
                                ps, xt_sb[kc][:, xo + tt * 128:xo + (tt + 1) * 128],
                                wv_sb[kc][:, half * VH:(half + 1) * VH],
                                start=(kc == 0), stop=False)
                        nc.tensor.matmul(
                            ps, ones_sb[:, tt * 128:(tt + 1) * 128],
                            wv_bias[:, half * VH:(half + 1) * VH],
                            start=False, stop=True)
                        nc.vector.tensor_copy(v[:, half * VH:(half + 1) * VH], ps)
                    v_sbs[b01][tt] = v
            return v_sbs

        def head_units(qk_b, v_b, ao_sb):
            pend = {}

            def head_front(h):
                qt = qk_b[h // 2]
                kt_c = qk_b[KC + h // 2]
                po = (h % 2) * 64
                st = psS.tile([128, 2 * TP], f32, tag="st", name="st")
                for kt in range(2):
                    KT = KT_SZ[kt]
                    nc.tensor.matmul(
                        st[0:KT, kt * TP:(kt + 1) * TP],
                        kt_c[po:po + 64, kt * 128:kt * 128 + KT],
                        qt[po:po + 64, :], start=(kt == 0), stop=(kt == 1))
                e = ep.tile([128, 2 * TP], f32, tag="e", name="e")
                nc.scalar.activation(e, st, Exp)
                pt = ptp.tile([128, 2 * TP], mdt, tag="pt", name="pt")
                nc.vector.tensor_mul(pt, e, erpb_sb[h])
                pend[h] = pt

            def head_back(h):
                po = (h % 2) * 64
                pt = pend.pop(h)
                ot = psO.tile([65, TP], f32, tag="ot", name="ot")
                for kt in range(2):
                    KT = KT_SZ[kt]
                    nc.tensor.matmul(
                        ot, v_b[kt][0:KT, h * 65:(h + 1) * 65],
                        pt[0:KT, kt * TP:(kt + 1) * TP],
                        start=(kt == 0), stop=(kt == 1))
                r_sb = lrp.tile([1, TP], f32, tag="r", name="r")
                nc.vector.reciprocal(r_sb, ot[64:65, :])
                r_full = rfp.tile([64, TP], f32, tag="rf", name="rf")
                nc.gpsimd.partition_broadcast(r_full, r_sb)
                nc.vector.tensor_mul(ao_sb[h // 2][po:po + 64, :],
                                     ot[0:64, :], r_full)

            def unit(h):
                if h < H:
                    head_front(h)
                if h >= 1:
                    head_back(h - 1)

            return [lambda h=h: unit(h) for h in range(H + 1)]

        def emit_proj(b, ao_sb):
            for tt in range(2):
                t0 = tt * 128
                tl = min(128, NT - t0)
                for half in range(2):
                    ps = psA.tile([128, PH], f32, tag="mm", name="mm")
                    for dc in range(KC):
                        nc.tensor.matmul(
                            ps[0:tl], ao_sb[dc][:, t0:t0 + tl],
                            pw_sb[dc][:, half * PH:(half + 1) * PH],
                            start=(dc == 0), stop=(dc == KC - 1))
                    y_sb = yp.tile([128, PH], f32, tag="y", name="y")
                    nc.vector.tensor_add(
                        y_sb[0:tl], ps[0:tl],
                        pb_full[0:tl, half * PH:(half + 1) * PH])
                    nc.sync.dma_start(
                        out=y_d[b, t0:t0 + tl, half * PH:(half + 1) * PH],
                        in_=y_sb[0:tl])

        def whole_pass():
            qk_cur, xt_cur, thunks = build_A(0)
            for t in thunks:
                t()
            for p in range(PAIRS):
                v_cur = emit_V(xt_cur)
                if p + 1 < PAIRS:
                    qk_nxt, xt_nxt, a_thunks = build_A(p + 1)
                else:
                    qk_nxt = xt_nxt = None
                    a_thunks = []
                ao_sbs = []
                for b01 in range(2):
                    ao_sb = [aop.tile([128, TP], mdt, tag=f"ao{dc}",
                                      name=f"ao{b01}_{dc}") for dc in range(KC)]
                    ao_sbs.append(ao_sb)
                emitted = 0
                n_units = 2 * (H + 1)
                for b01 in range(2):
                    units = head_units(qk_cur[b01], v_cur[b01], ao_sbs[b01])
                    for j, u in enumerate(units):
                        u()
                        i = b01 * (H + 1) + j
                        want = (i + 1) * len(a_thunks) // n_units
                        while emitted < want:
                            a_thunks[emitted]()
                            emitted += 1
                    if b01 == 0:
                        # b0's projection between the two head regions: dense,
                        # dependency-ready PE work adjacent to b1's chains
                        emit_proj(2 * p, ao_sbs[0])
                emit_proj(2 * p + 1, ao_sbs[1])
                qk_cur, xt_cur = qk_nxt, xt_nxt

        if PERF_REPS > 0:
            with tc.For_i(0, PERF_REPS, 1):
                whole_pass()
        else:
            whole_pass()

    nc.compile()
    _prog_cache[key] = nc
    return nc


def _host_prep(x, qkv_w, q_bias, v_bias, q_lora_a, q_lora_b, k_lora_a,
               k_lora_b, v_lora_a, v_lora_b, rel_pos_table, proj_w, proj_b,
               rel_pos_index):
    f = np.float32
    x = np.asarray(x, f)
    qkv_w = np.asarray(qkv_w, f)
    q_bias = np.asarray(q_bias, f)
    v_bias = np.asarray(v_bias, f)
    proj_w = np.asarray(proj_w, f)
    proj_b = np.asarray(proj_b, f)
    rel_pos_table = np.asarray(rel_pos_table, f)
    rel_pos_index = np.asarray(rel_pos_index)

    # fold LoRA (x @ A.T @ B.T == x @ (B@A).T) and attention scale into weights
    lora = np.vstack([
        np.asarray(q_lora_b, np.float64) @ np.asarray(q_lora_a, np.float64),
        np.asarray(k_lora_b, np.float64) @ np.asarray(k_lora_a, np.float64),
        np.asarray(v_lora_b, np.float64) @ np.asarray(v_lora_a, np.float64),
    ])
    W = (np.asarray(qkv_w, np.float64) + lora)
    W[0:C] *= SCALE
    W = W.astype(f)

    wqk = np.ascontiguousarray(W[0:2 * C].T)                     # [768, 1536]
    bqk = np.ascontiguousarray(
        np.concatenate([q_bias * SCALE, np.zeros(C, f)]).reshape(NCO, 128).T)

    WvT = W[2 * C:3 * C].T                                       # [768, 768]
    wv = np.zeros((C + 1, VW), f)
    for h in range(H):
        wv[0:C, h * 65:h * 65 + 64] = WvT[:, h * 64:(h + 1) * 64]
        wv[C, h * 65 + 64] = 1.0
    pw = np.ascontiguousarray(proj_w.T)
    # softmax weights sum to 1 -> v_bias adds a constant to attn_out;
    # fold it into the projection bias: pb = proj_b + proj_w @ v_bias
    pb = (proj_b + proj_w @ v_bias).reshape(1, C).astype(f)

    # exp(rpb) transposed per head: erpb[h, kt, k_row, q_col]
    rpb = rel_pos_table[rel_pos_index.reshape(-1).astype(np.int64)]
    rpb = rpb.reshape(NT, NT, H)                                  # [q, k, h]
    erpb_t = np.exp(rpb).transpose(2, 1, 0).astype(f)             # [h, k, q]
    erpb = np.ones((H, 2, 128, TP), f)
    erpb[:, 0, 0:128, 0:NT] = erpb_t[:, 0:128, :]
    erpb[:, 1, 0:NT - 128, 0:NT] = erpb_t[:, 128:NT, :]
    erpb = np.ascontiguousarray(np.concatenate([erpb[:, 0], erpb[:, 1]], axis=2))

    xt = np.zeros((B, C, TP), f)
    xt[:, :, 0:NT] = x.transpose(0, 2, 1)
    # pack batch pairs side by side: [B//2, C, 2*TP]
    xt = np.ascontiguousarray(
        xt.reshape(B // 2, 2, C, TP).transpose(0, 2, 1, 3).reshape(B // 2, C, 2 * TP))

    mnp = _mm_np()
    return {
        "xt": xt.astype(mnp),
        "wqk": wqk.astype(mnp),
        "bqk": bqk,
        "wv": wv.astype(mnp),
        "pw": pw.astype(mnp),
        "erpb": erpb.astype(np.float16),
        "pb": pb,
        "ones": np.ones((1, TP), mnp),
    }


def kernel(**inputs):
    arrs = _host_prep(**inputs)
    nc = _build_program()
    in_maps = []
    ppc = BPC // 2
    for ci in range(NCORES):
        m = dict(arrs)
        m["xt"] = np.ascontiguousarray(arrs["xt"][ci * ppc:(ci + 1) * ppc])
        in_maps.append(m)
    last_exc = None
    for attempt in range(3):
        try:
            res = bass_utils.run_bass_kernel_spmd(
                nc, in_maps, core_ids=list(range(NCORES)))
            break
        except Exception as e:  # transient NRT device flakes recover on retry
            last_exc = e
            import time
            time.sleep(5.0 * (attempt + 1))
    else:
        raise last_exc
    out = np.concatenate([r["y"] for r in res.results], axis=0)
    return out.astype(np.float32)



# revision 14
# speedup vs baseline: 1.1808x; 1.1808x over previous
"""Trainium2 Bass kernel for nn_Attention_LoRA (Swin-style attention w/ LoRA + rel-pos bias).

v2 strategy (vs v1 baseline at ~493us):
  - Data-parallel over batch: 64 batches -> 8 cores x 8 batches, pairs packed.
  - Host folds LoRA + scale into weights; v_bias folded into proj bias.
  - Rel-pos bias is injected ADDITIVELY into the score PSUM via an
    identity-stationary matmul (sets has_written), so softmax numerator is a
    single exp() on the Act engine -- no DVE multiply per head.
  - Softmax denominators are computed with ones-matrix [KT,64] matmuls that
    REPLICATE each head's denominator across 64 partitions for free (matmul
    cost is free-dim driven), two heads per PSUM bank (column-split; matmul
    dst must start at partition 0). One [64,512] reciprocal per head-pair
    (vs one [1,256] reciprocal + gpsimd broadcast per head) and zero GpSimd
    broadcasts.
  - Normalization muls write ao quadrants directly via DVE partition-shifted
    writes (in partitions 0:64 -> out 64:128), as the v1 kernel did.
  - proj bias applied via a ones-row matmul into the proj PSUM group; the
    y move is a plain copy.
  DVE op count per pass drops ~448 -> ~100 and Pool to zero; PE gains the
  ident/l matmuls (~+35us busy) but stays the only heavily-loaded engine.
"""
import os
from contextlib import ExitStack

import numpy as np

import concourse.bacc as bacc
import concourse.mybir as mybir
import concourse.tile as tile
import concourse.bass as bass
from concourse import bass_utils

B, NT, C, H, WS, RANK = 64, 197, 768, 12, 14, 24
HD = C // H
SCALE = HD ** -0.5
NCORES = 8
BPC = B // NCORES          # batches per core
TP = 256                   # padded token count
KC = C // 128              # 6 contraction chunks
NCO = (2 * C) // 128       # 12 q+k output chunks
PH = C // 2                # 384

MM_MODE = os.environ.get("MM_MODE", "f32r")   # f32r | bf16 | f32
PERF_REPS = int(os.environ.get("PERF_REPS", "0"))

_prog_cache = {}


def _mm_dt():
    return {
        "f32r": mybir.dt.float32r,
        "bf16": mybir.dt.bfloat16,
        "f32": mybir.dt.float32,
    }[MM_MODE]


def _mm_np():
    if MM_MODE == "bf16":
        import ml_dtypes
        return ml_dtypes.bfloat16
    return np.float32


def _bf16_np():
    import ml_dtypes
    return ml_dtypes.bfloat16


def _build_program():
    key = (MM_MODE, PERF_REPS)
    if key in _prog_cache:
        return _prog_cache[key]

    f32 = mybir.dt.float32
    bf16 = mybir.dt.bfloat16
    mdt = _mm_dt()
    Exp = mybir.ActivationFunctionType.Exp

    nc = bacc.Bacc("TRN2", target_bir_lowering=False, debug=False,
                   num_devices=NCORES)
    xt_d = nc.dram_tensor("xt", [BPC // 2, C, 2 * TP], mdt,
                          kind="ExternalInput").ap()
    wqk_d = nc.dram_tensor("wqk", [C, 2 * C], mdt, kind="ExternalInput").ap()
    bqk_d = nc.dram_tensor("bqk", [128, NCO], f32, kind="ExternalInput").ap()
    wv_d = nc.dram_tensor("wv", [C, C], mdt, kind="ExternalInput").ap()
    ones_d = nc.dram_tensor("ones", [1, TP], mdt, kind="ExternalInput").ap()
    onesk_d = nc.dram_tensor("onesk", [128, 64], mdt, kind="ExternalInput").ap()
    pw_d = nc.dram_tensor("pw", [C, C], mdt, kind="ExternalInput").ap()
    pb_d = nc.dram_tensor("pb", [1, C], mdt, kind="ExternalInput").ap()
    rpbt_d = nc.dram_tensor("rpbt", [H, 128, 2 * TP], bf16,
                            kind="ExternalInput").ap()
    ident_d = nc.dram_tensor("ident", [128, 128], bf16,
                             kind="ExternalInput").ap()
    y_d = nc.dram_tensor("y", [BPC, NT, C], f32, kind="ExternalOutput").ap()

    with tile.TileContext(nc) as tc, ExitStack() as ctx, \
            nc.allow_low_precision("fp32r/bf16 matmul inputs by design"):
        consts = ctx.enter_context(tc.tile_pool(name="consts", bufs=1))
        xp = ctx.enter_context(tc.tile_pool(name="xp", bufs=1))
        qkp = ctx.enter_context(tc.tile_pool(name="qkp", bufs=1))
        vp = ctx.enter_context(tc.tile_pool(name="vp", bufs=1))
        ptp = ctx.enter_context(tc.tile_pool(name="ptp", bufs=3))
        rfp = ctx.enter_context(tc.tile_pool(name="rfp", bufs=2))
        aop = ctx.enter_context(tc.tile_pool(name="aop", bufs=2))
        yp = ctx.enter_context(tc.tile_pool(name="yp", bufs=2))
        psA = ctx.enter_context(tc.tile_pool(name="psA", bufs=2, space="PSUM"))
        psS = ctx.enter_context(tc.tile_pool(name="psS", bufs=2, space="PSUM"))
        psO = ctx.enter_context(tc.tile_pool(name="psO", bufs=2, space="PSUM"))
        psR = ctx.enter_context(tc.tile_pool(name="psR", bufs=2, space="PSUM"))

        # ---- constants ----
        wqk_sb = []
        wv_sb = []
        pw_sb = []
        for kc in range(KC):
            t = consts.tile([128, 2 * C], mdt, tag=f"wqk{kc}")
            nc.sync.dma_start(out=t, in_=wqk_d[kc * 128:(kc + 1) * 128, :])
            wqk_sb.append(t)
        for kc in range(KC):
            t = consts.tile([128, C], mdt, tag=f"wv{kc}")
            nc.sync.dma_start(out=t, in_=wv_d[kc * 128:(kc + 1) * 128, :])
            wv_sb.append(t)
        for kc in range(KC):
            t = consts.tile([128, C], mdt, tag=f"pw{kc}")
            nc.sync.dma_start(out=t, in_=pw_d[kc * 128:(kc + 1) * 128, :])
            pw_sb.append(t)
        ones_sb = consts.tile([1, TP], mdt, tag="ones")
        nc.sync.dma_start(out=ones_sb, in_=ones_d)
        onesk_sb = consts.tile([128, 64], mdt, tag="onesk")
        nc.sync.dma_start(out=onesk_sb, in_=onesk_d)
        pb_sb = consts.tile([1, C], mdt, tag="pb")
        nc.sync.dma_start(out=pb_sb, in_=pb_d)
        bqk_sb = consts.tile([128, NCO], f32, tag="bqk")
        nc.sync.dma_start(out=bqk_sb, in_=bqk_d)
        ident_sb = consts.tile([128, 128], bf16, tag="ident")
        nc.sync.dma_start(out=ident_sb, in_=ident_d)
        rpbt_sb = {}
        for h in range(H):
            t = consts.tile([128, 2 * TP], bf16, tag=f"rpbt{h}")
            nc.sync.dma_start(out=t, in_=rpbt_d[h, :, :])
            rpbt_sb[h] = t

        KT_SZ = [128, NT - 128]  # 128, 69

        PAIRS = BPC // 2
        co_order = [c for pr in zip(range(KC), range(KC, NCO)) for c in pr]

        def build_A(p):
            """Emit pair p's xt DMAs now; return (qk_sb, xt_sb, thunks)."""
            par = p % 2
            xt_sb = []
            for kc in range(KC):
                t = xp.tile([128, 2 * TP], mdt, tag=f"x{par}_{kc}",
                            name=f"x{par}_{kc}")
                nc.sync.dma_start(out=t, in_=xt_d[p, kc * 128:(kc + 1) * 128, :])
                xt_sb.append(t)
            qk_sb = [[None] * NCO, [None] * NCO]

            def qk_thunk(co):
                ps = psA.tile([128, 2 * TP], f32, tag="mm", name="mm")
                for kc in range(KC):
                    nc.tensor.matmul(
                        ps, wqk_sb[kc][:, co * 128:(co + 1) * 128], xt_sb[kc],
                        start=(kc == 0), stop=(kc == KC - 1))
                for b01 in range(2):
                    qk = qkp.tile([128, TP], mdt, tag=f"qk{par}_{b01}_{co}",
                                  name=f"qk{par}_{b01}_{co}")
                    nc.scalar.add(qk, ps[:, b01 * TP:(b01 + 1) * TP],
                                  add=bqk_sb[:, co:co + 1])
                    qk_sb[b01][co] = qk

            thunks = [lambda co=co: qk_thunk(co) for co in co_order]
            return qk_sb, xt_sb, thunks

        def emit_V(xt_sb):
            """Both batches' v passes (token-major [tok, C]) for a pair."""
            v_sbs = [[None, None], [None, None]]
            for b01 in range(2):
                xo = b01 * TP
                for tt in range(2):
                    v = vp.tile([128, C], mdt, tag=f"v{b01}_{tt}",
                                name=f"v{b01}_{tt}")
                    for half in range(2):
                        ps = psA.tile([128, PH], f32, tag="mm", name="mm")
                        for kc in range(KC):
                            nc.tensor.matmul(
                                ps, xt_sb[kc][:, xo + tt * 128:xo + (tt + 1) * 128],
                                wv_sb[kc][:, half * PH:(half + 1) * PH],
                                start=(kc == 0), stop=(kc == KC - 1))
                        nc.vector.tensor_copy(v[:, half * PH:(half + 1) * PH], ps)
                    v_sbs[b01][tt] = v
            return v_sbs

        def head_units(qk_b, v_b, ao_sb):
            """Units 0..H. unit(h): front(h) + back(h-1) (+ pair tail after
            each odd head's back). ao_sb: list of 6 [128, TP] sbuf tiles the
            tails fill in (created lazily here)."""
            pend = {}
            cur = {}

            def head_front(h):
                qt = qk_b[h // 2]
                kt_c = qk_b[KC + h // 2]
                po = (h % 2) * 64
                st = psS.tile([128, 2 * TP], f32, tag="st", name="st")
                nc.tensor.matmul(st, ident_sb, rpbt_sb[h],
                                 start=True, stop=False)
                for kt in range(2):
                    KT = KT_SZ[kt]
                    nc.tensor.matmul(
                        st[0:KT, kt * TP:(kt + 1) * TP],
                        kt_c[po:po + 64, kt * 128:kt * 128 + KT],
                        qt[po:po + 64, :], start=False, stop=(kt == 1))
                pt = ptp.tile([128, 2 * TP], mdt, tag="pt", name="pt")
                nc.scalar.activation(pt, st, Exp)
                pend[h] = pt

            def head_back(h):
                ch = h % 2          # column half within the pair bank
                pt = pend.pop(h)
                if ch == 0:
                    cur["ot"] = psO.tile([64, 2 * TP], f32, tag="ot",
                                         name=f"ot{h // 2}")
                    cur["lr"] = psR.tile([64, 2 * TP], f32, tag="lr",
                                         name=f"lr{h // 2}")
                ot2, lr2 = cur["ot"], cur["lr"]
                for kt in range(2):
                    KT = KT_SZ[kt]
                    nc.tensor.matmul(
                        ot2[:, ch * TP:(ch + 1) * TP],
                        v_b[kt][0:KT, h * 64:(h + 1) * 64],
                        pt[0:KT, kt * TP:(kt + 1) * TP],
                        start=(kt == 0), stop=(kt == 1))
                # denominator, replicated across 64 partitions by the matmul
                for kt in range(2):
                    KT = KT_SZ[kt]
                    nc.tensor.matmul(
                        lr2[:, ch * TP:(ch + 1) * TP],
                        onesk_sb[0:KT, :],
                        pt[0:KT, kt * TP:(kt + 1) * TP],
                        start=(kt == 0), stop=(kt == 1))

            def pair_tail(dc):
                ot2, lr2 = cur["ot"], cur["lr"]
                rf = rfp.tile([64, 2 * TP], mybir.dt.float32, tag="rf",
                              name=f"rf{dc}")
                nc.vector.reciprocal(rf, lr2)
                ao = aop.tile([128, TP], mdt, tag=f"aosb{dc}",
                              name=f"aosb{dc}")
                nc.vector.tensor_mul(ao[0:64, :], ot2[:, 0:TP], rf[:, 0:TP])
                nc.vector.tensor_mul(ao[64:128, :], ot2[:, TP:2 * TP],
                                     rf[:, TP:2 * TP])
                ao_sb.append(ao)

            def unit(h):
                if h < H:
                    head_front(h)
                if h >= 1:
                    head_back(h - 1)
                if h >= 2 and h % 2 == 0:
                    pair_tail(h // 2 - 1)

            return [lambda h=h: unit(h) for h in range(H + 1)]

        def proj_quarters(b, ao_sb):
            """Return 4 thunks, each one quarter of the projection."""
            def quarter(tt, half):
                t0 = tt * 128
                tl = min(128, NT - t0)
                ps = psA.tile([128, PH], f32, tag="mm", name="mm")
                for dc in range(KC):
                    nc.tensor.matmul(
                        ps[0:tl], ao_sb[dc][:, t0:t0 + tl],
                        pw_sb[dc][:, half * PH:(half + 1) * PH],
                        start=(dc == 0), stop=False)
                nc.tensor.matmul(
                    ps[0:tl], ones_sb[:, 0:tl],
                    pb_sb[:, half * PH:(half + 1) * PH],
                    start=False, stop=True)
                y_sb = yp.tile([128, PH], f32, tag="y", name="y")
                nc.vector.tensor_copy(y_sb[0:tl], ps[0:tl])
                nc.sync.dma_start(
                    out=y_d[b, t0:t0 + tl, half * PH:(half + 1) * PH],
                    in_=y_sb[0:tl])

            return [lambda tt=tt, half=half: quarter(tt, half)
                    for tt in range(2) for half in range(2)]

        def run_units(units, fillers, fill_from=1):
            """Run units, spreading fillers across units[fill_from:]."""
            done = 0
            span = len(units) - fill_from
            for i, u in enumerate(units):
                u()
                if i + 1 <= fill_from:
                    continue
                want = (i + 1 - fill_from) * len(fillers) // span
                while done < want:
                    fillers[done]()
                    done += 1
            while done < len(fillers):
                fillers[done]()
                done += 1

        def whole_pass():
            qk_cur, xt_cur, thunks = build_A(0)
            for t in thunks:
                t()
            v_cur = emit_V(xt_cur)
            for p in range(PAIRS):
                last = p + 1 >= PAIRS
                if not last:
                    qk_nxt, xt_nxt, a_thunks = build_A(p + 1)
                else:
                    qk_nxt = xt_nxt = None
                    a_thunks = []

                # batch 0
                ao0 = []
                units0 = head_units(qk_cur[0], v_cur[0], ao0)
                run_units(units0, a_thunks[:6])

                # batch 1; fill with remaining thunks then b0's projection
                ao1 = []
                units1 = head_units(qk_cur[1], v_cur[1], ao1)
                run_units(units1, a_thunks[6:] + proj_quarters(2 * p, ao0),
                          fill_from=3)

                if not last:
                    v_nxt = emit_V(xt_nxt)   # PE filler under b1's tail chain
                else:
                    v_nxt = None
                for q in proj_quarters(2 * p + 1, ao1):
                    q()
                qk_cur, xt_cur, v_cur = qk_nxt, xt_nxt, v_nxt

        if PERF_REPS > 0:
            with tc.For_i(0, PERF_REPS, 1):
                whole_pass()
        else:
            whole_pass()

    nc.compile()
    _prog_cache[key] = nc
    return nc


def _host_prep(x, qkv_w, q_bias, v_bias, q_lora_a, q_lora_b, k_lora_a,
               k_lora_b, v_lora_a, v_lora_b, rel_pos_table, proj_w, proj_b,
               rel_pos_index):
    f = np.float32
    x = np.asarray(x, f)
    qkv_w = np.asarray(qkv_w, f)
    q_bias = np.asarray(q_bias, f)
    v_bias = np.asarray(v_bias, f)
    proj_w = np.asarray(proj_w, f)
    proj_b = np.asarray(proj_b, f)
    rel_pos_table = np.asarray(rel_pos_table, f)
    rel_pos_index = np.asarray(rel_pos_index)

    # fold LoRA (x @ A.T @ B.T == x @ (B@A).T) and attention scale into weights
    lora = np.vstack([
        np.asarray(q_lora_b, np.float64) @ np.asarray(q_lora_a, np.float64),
        np.asarray(k_lora_b, np.float64) @ np.asarray(k_lora_a, np.float64),
        np.asarray(v_lora_b, np.float64) @ np.asarray(v_lora_a, np.float64),
    ])
    W = (np.asarray(qkv_w, np.float64) + lora)
    W[0:C] *= SCALE
    W = W.astype(f)

    wqk = np.ascontiguousarray(W[0:2 * C].T)                     # [768, 1536]
    bqk = np.ascontiguousarray(
        np.concatenate([q_bias * SCALE, np.zeros(C, f)]).reshape(NCO, 128).T)
    wv = np.ascontiguousarray(W[2 * C:3 * C].T)                  # [768, 768]
    pw = np.ascontiguousarray(proj_w.T)
    # softmax weights sum to 1 -> v_bias adds a constant to attn_out;
    # fold it into the projection bias: pb = proj_b + proj_w @ v_bias
    pb = (proj_b + proj_w @ v_bias).reshape(1, C).astype(f)

    # raw rel-pos bias transposed per head: rpbt[h, k_row, kt*TP + q_col]
    rpb = rel_pos_table[rel_pos_index.reshape(-1).astype(np.int64)]
    rpb = rpb.reshape(NT, NT, H)                                  # [q, k, h]
    rpb_t = rpb.transpose(2, 1, 0).astype(f)                      # [h, k, q]
    rpbt = np.zeros((H, 2, 128, TP), f)
    rpbt[:, 0, 0:128, 0:NT] = rpb_t[:, 0:128, :]
    rpbt[:, 1, 0:NT - 128, 0:NT] = rpb_t[:, 128:NT, :]
    rpbt = np.ascontiguousarray(
        np.concatenate([rpbt[:, 0], rpbt[:, 1]], axis=2))         # [H,128,512]

    xt = np.zeros((B, C, TP), f)
    xt[:, :, 0:NT] = x.transpose(0, 2, 1)
    # pack batch pairs side by side: [B//2, C, 2*TP]
    xt = np.ascontiguousarray(
        xt.reshape(B // 2, 2, C, TP).transpose(0, 2, 1, 3).reshape(B // 2, C, 2 * TP))

    mnp = _mm_np()
    bf = _bf16_np()
    return {
        "xt": xt.astype(mnp),
        "wqk": wqk.astype(mnp),
        "bqk": bqk,
        "wv": wv.astype(mnp),
        "pw": pw.astype(mnp),
        "pb": pb.astype(mnp),
        "ones": np.ones((1, TP), mnp),
        "onesk": np.ones((128, 64), mnp),
        "rpbt": rpbt.astype(bf),
        "ident": np.eye(128, dtype=np.float32).astype(bf),
    }


def kernel(**inputs):
    arrs = _host_prep(**inputs)
    nc = _build_program()
    in_maps = []
    ppc = BPC // 2
    for ci in range(NCORES):
        m = dict(arrs)
        m["xt"] = np.ascontiguousarray(arrs["xt"][ci * ppc:(ci + 1) * ppc])
        in_maps.append(m)
    last_exc = None
    for attempt in range(3):
        try:
            res = bass_utils.run_bass_kernel_spmd(
                nc, in_maps, core_ids=list(range(NCORES)))
            break
        except Exception as e:  # transient NRT device flakes recover on retry
            last_exc = e
            import time
            time.sleep(5.0 * (attempt + 1))
    else:
        raise last_exc
    out = np.concatenate([r["y"] for r in res.results], axis=0)
    return out.astype(np.float32)


# revision 18
# speedup vs baseline: 1.7807x; 1.5080x over previous
"""Trainium2 Bass kernel for nn_Attention_LoRA (Swin-style attention w/ LoRA + rel-pos bias).

Strategy:
  - Data-parallel over batch: 64 batches -> 8 cores x 8 batches.
  - Host-side prep (pure layout / constant folding, no per-token compute):
      * LoRA adapters folded into qkv weight:  W_eff = qkv_w + B @ A  (per q/k/v)
      * attention scale folded into q weights & bias
      * weights pre-transposed; v-weights head-interleaved with a ones column so
        the PV matmul also produces the softmax denominator
      * rel-pos bias gathered + exponentiated: softmax(S+rpb) uses exp(S)*exp(rpb)
      * x pre-transposed to [B, C, N] and token-padded to 256
  - On-device, per batch, everything stays "transposed" (features/keys on
    partitions) so NO on-chip transposes are needed:
      qkT[c,t]  = Wqk @ xT                       (12x6 matmuls, fp32r)
      v[t,c']   = x @ WvT_ext                    (token-major, ones col + bias row)
      ST[k,q]   = kT.T @ qT                      (per head, keys on partitions)
      PT[k,q]   = exp(ST) * exp_rpbT[h]
      OT_ext    = v_ext.T @ PT   -> rows 0:64 = unnormalized out, row 64 = sum_k P
      aoT       = OT[0:64] * (1/l)  (GpSimd partition-broadcast of 1/l)
      y[t,c]    = aoT.T @ projWT + (proj_b + proj_w @ v_bias)
  Measured on trn2 (8 cores, axon): ~490-520 us absolute (direct method) per
  core for its 8-batch share (paired A/B ordering: 88 us faster than the v7
  variant of this kernel),
  scaled absmax error vs fp32 reference ~2.9e-4 (fp32r matmuls).
"""
import os
from contextlib import ExitStack

import numpy as np

import concourse.bacc as bacc
import concourse.mybir as mybir
import concourse.tile as tile
from concourse import bass_utils

B, NT, C, H, WS, RANK = 64, 197, 768, 12, 14, 24
HD = C // H
SCALE = HD ** -0.5
NCORES = 8
BPC = B // NCORES          # batches per core
TP = 256                   # padded token count
KC = C // 128              # 6 contraction chunks
NCO = (2 * C) // 128       # 12 q+k output chunks
VW = H * (HD + 1)          # 780: v head-blocks of 65 (64 v cols + ones col)
VH = VW // 2               # 390
PH = C // 2                # 384

MM_MODE = os.environ.get("MM_MODE", "bf16")   # f32r | bf16 | f32
PERF_REPS = int(os.environ.get("PERF_REPS", "0"))
UNROLL = int(os.environ.get("UNROLL", "2"))

_prog_cache = {}


def _mm_dt():
    return {
        "f32r": mybir.dt.float32r,
        "bf16": mybir.dt.bfloat16,
        "f32": mybir.dt.float32,
    }[MM_MODE]


def _mm_np():
    if MM_MODE == "bf16":
        import ml_dtypes
        return ml_dtypes.bfloat16
    return np.float32


def _bf16():
    import ml_dtypes
    return ml_dtypes.bfloat16


def _build_program():
    key = (MM_MODE, PERF_REPS, UNROLL)
    if key in _prog_cache:
        return _prog_cache[key]

    f32 = mybir.dt.float32
    mdt = _mm_dt()
    Exp = mybir.ActivationFunctionType.Exp

    nc = bacc.Bacc("TRN2", target_bir_lowering=False, debug=False,
                   num_devices=NCORES)
    xt_d = nc.dram_tensor("xt", [BPC // 2, C, 2 * TP], mdt,
                          kind="ExternalInput").ap()
    wqk_d = nc.dram_tensor("wqk", [C, 2 * C], mdt, kind="ExternalInput").ap()
    bqk_d = nc.dram_tensor("bqk", [128, NCO], f32, kind="ExternalInput").ap()
    wv_d = nc.dram_tensor("wv", [C + 1, VW], mdt, kind="ExternalInput").ap()
    ones_d = nc.dram_tensor("ones", [1, TP], mdt, kind="ExternalInput").ap()
    pw_d = nc.dram_tensor("pw", [C, C], mdt, kind="ExternalInput").ap()
    pb_d = nc.dram_tensor("pb", [1, C], f32, kind="ExternalInput").ap()
    erpb_d = nc.dram_tensor("erpb", [H, 128, 2 * TP], mybir.dt.bfloat16,
                            kind="ExternalInput").ap()
    y_d = nc.dram_tensor("y", [BPC, NT, C], f32, kind="ExternalOutput").ap()

    with tile.TileContext(nc) as tc, ExitStack() as ctx, \
            nc.allow_low_precision("fp32r/bf16 matmul inputs by design"):
        consts = ctx.enter_context(tc.tile_pool(name="consts", bufs=1))
        xp = ctx.enter_context(tc.tile_pool(name="xp", bufs=1))
        qkp = ctx.enter_context(tc.tile_pool(name="qkp", bufs=1))
        vp = ctx.enter_context(tc.tile_pool(name="vp", bufs=1))
        ep = ctx.enter_context(tc.tile_pool(name="ep", bufs=2))
        ptp = ctx.enter_context(tc.tile_pool(name="ptp", bufs=3))
        lrp = ctx.enter_context(tc.tile_pool(name="lrp", bufs=3))
        rfp = ctx.enter_context(tc.tile_pool(name="rfp", bufs=3))
        aop = ctx.enter_context(tc.tile_pool(name="aop", bufs=2))
        yp = ctx.enter_context(tc.tile_pool(name="yp", bufs=2))
        psA = ctx.enter_context(tc.tile_pool(name="psA", bufs=2, space="PSUM"))
        psS = ctx.enter_context(tc.tile_pool(name="psS", bufs=3, space="PSUM"))
        psO = ctx.enter_context(tc.tile_pool(name="psO", bufs=3, space="PSUM"))

        # ---- constants ----
        wqk_sb = []
        wv_sb = []
        pw_sb = []
        for kc in range(KC):
            t = consts.tile([128, 2 * C], mdt, tag=f"wqk{kc}")
            nc.sync.dma_start(out=t, in_=wqk_d[kc * 128:(kc + 1) * 128, :])
            wqk_sb.append(t)
        for kc in range(KC):
            t = consts.tile([128, VW], mdt, tag=f"wv{kc}")
            nc.sync.dma_start(out=t, in_=wv_d[kc * 128:(kc + 1) * 128, :])
            wv_sb.append(t)
        for kc in range(KC):
            t = consts.tile([128, C], mdt, tag=f"pw{kc}")
            nc.sync.dma_start(out=t, in_=pw_d[kc * 128:(kc + 1) * 128, :])
            pw_sb.append(t)
        wv_bias = consts.tile([1, VW], mdt, tag="wvb")
        nc.sync.dma_start(out=wv_bias, in_=wv_d[C:C + 1, :])
        ones_sb = consts.tile([1, TP], mdt, tag="ones")
        nc.sync.dma_start(out=ones_sb, in_=ones_d)
        pb_full = consts.tile([128, C], f32, tag="pbf")
        import concourse.bass as bass
        nc.sync.dma_start(out=pb_full, in_=bass.AP(
            tensor=pb_d.tensor, offset=pb_d.offset,
            ap=[[0, 128]] + list(pb_d.ap[1:])))
        bqk_sb = consts.tile([128, NCO], f32, tag="bqk")
        nc.sync.dma_start(out=bqk_sb, in_=bqk_d)
        erpb_sb = {}
        for h in range(H):
            t = consts.tile([128, 2 * TP], mybir.dt.bfloat16, tag=f"erpb{h}")
            nc.sync.dma_start(out=t, in_=erpb_d[h, :, :])
            erpb_sb[h] = t

        KT_SZ = [128, NT - 128]  # 128, 69

        PAIRS = BPC // 2
        co_order = [c for pr in zip(range(KC), range(KC, NCO)) for c in pr]

        def build_A(p):
            """Emit pair p's xt DMAs now; return (qk_sb, v_sbs, thunks).
            Each thunk emits one qk or v matmul group when invoked, so the
            caller can spread pair p's dense groups between the previous
            pair's latency-bound attention units. Tiles use pair-parity tag
            namespaces (bufs=1 per tag) so consecutive pairs never rotate
            through the same slot."""
            par = p % 2
            xt_sb = []
            for kc in range(KC):
                t = xp.tile([128, 2 * TP], mdt, tag=f"x{par}_{kc}",
                            name=f"x{par}_{kc}")
                nc.sync.dma_start(out=t, in_=xt_d[p, kc * 128:(kc + 1) * 128, :])
                xt_sb.append(t)
            qk_sb = [[None] * NCO, [None] * NCO]

            def qk_thunk(co):
                ps = psA.tile([128, 2 * TP], f32, tag="mm", name="mm")
                for kc in range(KC):
                    nc.tensor.matmul(
                        ps, wqk_sb[kc][:, co * 128:(co + 1) * 128], xt_sb[kc],
                        start=(kc == 0), stop=(kc == KC - 1))
                for b01 in range(2):
                    qk = qkp.tile([128, TP], mdt, tag=f"qk{par}_{b01}_{co}",
                                  name=f"qk{par}_{b01}_{co}")
                    nc.scalar.add(qk, ps[:, b01 * TP:(b01 + 1) * TP],
                                  add=bqk_sb[:, co:co + 1])
                    qk_sb[b01][co] = qk

            thunks = [lambda co=co: qk_thunk(co) for co in co_order]
            return qk_sb, xt_sb, thunks

        def emit_V(xt_sb):
            """Emit both batches' v passes (token-major) for the current pair."""
            v_sbs = [[None, None], [None, None]]
            for b01 in range(2):
                xo = b01 * TP
                for tt in range(2):
                    v = vp.tile([128, VW], mdt, tag=f"v{b01}_{tt}",
                                name=f"v{b01}_{tt}")
                    for half in range(2):
                        ps = psA.tile([128, VH], f32, tag="mm", name="mm")
                        for kc in range(KC):
                            nc.tensor.matmul(
                                ps, xt_sb[kc][:, xo + tt * 128:xo + (tt + 1) * 128],
                                wv_sb[kc][:, half * VH:(half + 1) * VH],
                                start=(kc == 0), stop=False)
                        nc.tensor.matmul(
                            ps, ones_sb[:, tt * 128:(tt + 1) * 128],
                            wv_bias[:, half * VH:(half + 1) * VH],
                            start=False, stop=True)
                        nc.vector.tensor_copy(v[:, half * VH:(half + 1) * VH], ps)
                    v_sbs[b01][tt] = v
            return v_sbs

        def head_units(qk_b, v_b, ao_sb):
            pend = {}

            def head_front(h):
                qt = qk_b[h // 2]
                kt_c = qk_b[KC + h // 2]
                po = (h % 2) * 64
                st = psS.tile([128, 2 * TP], f32, tag="st", name="st")
                for kt in range(2):
                    KT = KT_SZ[kt]
                    nc.tensor.matmul(
                        st[0:KT, kt * TP:(kt + 1) * TP],
                        kt_c[po:po + 64, kt * 128:kt * 128 + KT],
                        qt[po:po + 64, :], start=(kt == 0), stop=(kt == 1))
                e = ep.tile([128, 2 * TP], mybir.dt.bfloat16, tag="e", name="e")
                nc.scalar.activation(e, st, Exp)
                pt = ptp.tile([128, 2 * TP], mdt, tag="pt", name="pt")
                nc.vector.tensor_mul(pt, e, erpb_sb[h])
                pend[h] = pt

            def head_back(h):
                po = (h % 2) * 64
                pt = pend.pop(h)
                ot = psO.tile([65, TP], f32, tag="ot", name="ot")
                for kt in range(2):
                    KT = KT_SZ[kt]
                    nc.tensor.matmul(
                        ot, v_b[kt][0:KT, h * 65:(h + 1) * 65],
                        pt[0:KT, kt * TP:(kt + 1) * TP],
                        start=(kt == 0), stop=(kt == 1))
                r_sb = lrp.tile([1, TP], f32, tag="r", name="r")
                nc.vector.reciprocal(r_sb, ot[64:65, :])
                r_full = rfp.tile([64, TP], f32, tag="rf", name="rf")
                nc.gpsimd.partition_broadcast(r_full, r_sb)
                nc.vector.tensor_mul(ao_sb[h // 2][po:po + 64, :],
                                     ot[0:64, :], r_full)

            def unit(h):
                if h < H:
                    head_front(h)
                if h >= 1:
                    head_back(h - 1)

            return [lambda h=h: unit(h) for h in range(H + 1)]

        def emit_proj(b, ao_sb):
            for tt in range(2):
                t0 = tt * 128
                tl = min(128, NT - t0)
                for half in range(2):
                    ps = psA.tile([128, PH], f32, tag="mm", name="mm")
                    for dc in range(KC):
                        nc.tensor.matmul(
                            ps[0:tl], ao_sb[dc][:, t0:t0 + tl],
                            pw_sb[dc][:, half * PH:(half + 1) * PH],
                            start=(dc == 0), stop=(dc == KC - 1))
                    y_sb = yp.tile([128, PH], f32, tag="y", name="y")
                    nc.vector.tensor_add(
                        y_sb[0:tl], ps[0:tl],
                        pb_full[0:tl, half * PH:(half + 1) * PH])
                    nc.sync.dma_start(
                        out=y_d[b, t0:t0 + tl, half * PH:(half + 1) * PH],
                        in_=y_sb[0:tl])

        def whole_pass():
            qk_cur, xt_cur, thunks = build_A(0)
            for t in thunks:
                t()
            for p in range(PAIRS):
                v_cur = emit_V(xt_cur)
                if p + 1 < PAIRS:
                    qk_nxt, xt_nxt, a_thunks = build_A(p + 1)
                else:
                    qk_nxt = xt_nxt = None
                    a_thunks = []
                ao_sbs = []
                for b01 in range(2):
                    ao_sb = [aop.tile([128, TP], mdt, tag=f"ao{dc}",
                                      name=f"ao{b01}_{dc}") for dc in range(KC)]
                    ao_sbs.append(ao_sb)
                emitted = 0
                n_units = 2 * (H + 1)
                for b01 in range(2):
                    units = head_units(qk_cur[b01], v_cur[b01], ao_sbs[b01])
                    for j, u in enumerate(units):
                        u()
                        i = b01 * (H + 1) + j
                        want = (i + 1) * len(a_thunks) // n_units
                        while emitted < want:
                            a_thunks[emitted]()
                            emitted += 1
                    if b01 == 0:
                        # b0's projection between the two head regions: dense,
                        # dependency-ready PE work adjacent to b1's chains
                        emit_proj(2 * p, ao_sbs[0])
                emit_proj(2 * p + 1, ao_sbs[1])
                qk_cur, xt_cur = qk_nxt, xt_nxt

        if PERF_REPS > 0:
            unroll = UNROLL if PERF_REPS % UNROLL == 0 else 1
            with tc.For_i(0, PERF_REPS // unroll, 1):
                for _ in range(unroll):
                    whole_pass()
        else:
            whole_pass()

    nc.compile()
    _prog_cache[key] = nc
    return nc


def _host_prep(x, qkv_w, q_bias, v_bias, q_lora_a, q_lora_b, k_lora_a,
               k_lora_b, v_lora_a, v_lora_b, rel_pos_table, proj_w, proj_b,
               rel_pos_index):
    f = np.float32
    x = np.asarray(x, f)
    qkv_w = np.asarray(qkv_w, f)
    q_bias = np.asarray(q_bias, f)
    v_bias = np.asarray(v_bias, f)
    proj_w = np.asarray(proj_w, f)
    proj_b = np.asarray(proj_b, f)
    rel_pos_table = np.asarray(rel_pos_table, f)
    rel_pos_index = np.asarray(rel_pos_index)

    # fold LoRA (x @ A.T @ B.T == x @ (B@A).T) and attention scale into weights
    lora = np.vstack([
        np.asarray(q_lora_b, np.float64) @ np.asarray(q_lora_a, np.float64),
        np.asarray(k_lora_b, np.float64) @ np.asarray(k_lora_a, np.float64),
        np.asarray(v_lora_b, np.float64) @ np.asarray(v_lora_a, np.float64),
    ])
    W = (np.asarray(qkv_w, np.float64) + lora)
    W[0:C] *= SCALE
    W = W.astype(f)

    wqk = np.ascontiguousarray(W[0:2 * C].T)                     # [768, 1536]
    bqk = np.ascontiguousarray(
        np.concatenate([q_bias * SCALE, np.zeros(C, f)]).reshape(NCO, 128).T)

    WvT = W[2 * C:3 * C].T                                       # [768, 768]
    wv = np.zeros((C + 1, VW), f)
    for h in range(H):
        wv[0:C, h * 65:h * 65 + 64] = WvT[:, h * 64:(h + 1) * 64]
        wv[C, h * 65 + 64] = 1.0
    pw = np.ascontiguousarray(proj_w.T)
    # softmax weights sum to 1 -> v_bias adds a constant to attn_out;
    # fold it into the projection bias: pb = proj_b + proj_w @ v_bias
    pb = (proj_b + proj_w @ v_bias).reshape(1, C).astype(f)

    # exp(rpb) transposed per head: erpb[h, kt, k_row, q_col]
    rpb = rel_pos_table[rel_pos_index.reshape(-1).astype(np.int64)]
    rpb = rpb.reshape(NT, NT, H)                                  # [q, k, h]
    erpb_t = np.exp(rpb).transpose(2, 1, 0).astype(f)             # [h, k, q]
    erpb = np.ones((H, 2, 128, TP), f)
    erpb[:, 0, 0:128, 0:NT] = erpb_t[:, 0:128, :]
    erpb[:, 1, 0:NT - 128, 0:NT] = erpb_t[:, 128:NT, :]
    erpb = np.ascontiguousarray(np.concatenate([erpb[:, 0], erpb[:, 1]], axis=2))

    xt = np.zeros((B, C, TP), f)
    xt[:, :, 0:NT] = x.transpose(0, 2, 1)
    # pack batch pairs side by side: [B//2, C, 2*TP]
    xt = np.ascontiguousarray(
        xt.reshape(B // 2, 2, C, TP).transpose(0, 2, 1, 3).reshape(B // 2, C, 2 * TP))

    mnp = _mm_np()
    return {
        "xt": xt.astype(mnp),
        "wqk": wqk.astype(mnp),
        "bqk": bqk,
        "wv": wv.astype(mnp),
        "pw": pw.astype(mnp),
        "erpb": erpb.astype(_bf16()),
        "pb": pb,
        "ones": np.ones((1, TP), mnp),
    }


def kernel(**inputs):
    arrs = _host_prep(**inputs)
    nc = _build_program()
    in_maps = []
    ppc = BPC // 2
    for ci in range(NCORES):
        m = dict(arrs)
        m["xt"] = np.ascontiguousarray(arrs["xt"][ci * ppc:(ci + 1) * ppc])
        in_maps.append(m)
    last_exc = None
    for attempt in range(3):
        try:
            res = bass_utils.run_bass_kernel_spmd(
                nc, in_maps, core_ids=list(range(NCORES)))
            break
        except Exception as e:  # transient NRT device flakes recover on retry
            last_exc = e
            import time
            time.sleep(5.0 * (attempt + 1))
    else:
        raise last_exc
    out = np.concatenate([r["y"] for r in res.results], axis=0)
    return out.astype(np.float32)

